# revision 42
# baseline (speedup 1.0000x reference)
"""Trainium2 Bass kernel for ContinuousAttention (self-keyed RoPE attention,
strictly-causal masked scores, no softmax).

Reference computation (B=2, NH=16, T=2048, N=256, fp32):
    QR = rope(Q)                      # interleaved-pair RoPE, freqs quantized in pairs
    S  = QR @ QR^T                    # per (b, h); K input is unused by the module
    O  = (S * strict_causal_mask) @ V

Sharding: 32 (b*nh) heads over 8 NeuronCores, 4 heads per core; no
communication.  Each core runs an identical program on its head slice.

v4 design — chunked linear attention (fp16 operands, fp32 PSUM):
  With no softmax the masked product is exactly linear attention, so the
  strictly-causal past factorizes through a running state
      M_D = sum_{s < 256*D} QR_s V_s^T          (256 x 256, fp32 in PSUM)
  and per 256-row macro-chunk D (computed directly in O^T orientation):
      O_D^T = V_D^T @ strips_D      (intra: masked S^T blocks, 128-blocked)
            + M_sb^T-slices @ QR_D^T  (inter: per n2-block b, n1-chunk c,
                                       lhsT = M_sb[c][:, b], rhs = qct)
      M    += QR_D^T @ V_D            (persistent PSUM accumulation)
  This is algebraically exact and cuts matmul work ~2.5x vs materializing
  all causal score blocks.  RoPE is elementwise preprocessing and is done
  on the host; the host ships QR in both (n, t) and natural layouts plus V
  pre-arranged for fat DMA lines.  Output is O^T (h, n, t) fp16; the host
  transposes back.
"""

import math
import sys

import numpy as np

if "/opt/trn_rl_repo" not in sys.path:
    sys.path.insert(0, "/opt/trn_rl_repo")

import concourse.bass as bass
import concourse.mybir as mybir
import concourse.tile as tile
from concourse.bass_utils import run_bass_kernel_spmd

B, NH, T, N = 2, 16, 2048, 256
THETA = 2 ** 16
N_CORES = 8
H_PER_CORE = (B * NH) // N_CORES

F32 = mybir.dt.float32
FP16 = mybir.dt.float16
MULT = mybir.AluOpType.mult
HF = np.float16


def _split_overloaded_waits(nc, max_waits=1):
    """walrus in this container rejects >1 sync-wait per instruction; move
    extra waits onto preceding same-engine NoOps (semantically identical)."""
    n_split = 0
    for f in nc.m.functions:
        for bb in f.blocks:
            new_list = []
            changed = False
            for ins in bb.instructions:
                si = getattr(ins, "sync_info", None)
                if si is not None and len(si.on_wait) > max_waits:
                    waits = list(si.on_wait)
                    extra, keep = waits[:-max_waits], waits[-max_waits:]
                    k = 0
                    while extra:
                        chunk, extra = extra[:max_waits], extra[max_waits:]
                        nop = mybir.InstNoOp(
                            name=f"{ins.name}_wsplit{k}", ins=[], outs=[]
                        )
                        nop.engine = ins.engine
                        nop.sync_info = mybir.SyncInfo(on_wait=chunk, on_update=[])
                        new_list.append(nop)
                        k += 1
                    ins.sync_info = mybir.SyncInfo(
                        on_wait=keep, on_update=list(si.on_update)
                    )
                    changed = True
                    n_split += 1
                new_list.append(ins)
            if changed:
                bb.instructions = new_list
    return n_split


def rope_tables(t=T, n=N, dtype=np.float64):
    """cos table and sign-folded sin table, natural (t, n) layout."""
    idx = np.floor(np.arange(n, dtype=dtype) / dtype(2.0)) * dtype(2.0)
    freqs = (
        dtype(1.0) / (dtype(THETA) ** (idx / dtype(n))) / dtype(2.0 * math.pi)
    ).astype(dtype)
    phases = np.arange(t, dtype=dtype)[:, None] * freqs[None, :]
    ph = (phases % dtype(1.0)) * dtype(2.0 * math.pi)
    cos = np.cos(ph).astype(dtype)
    sin = np.sin(ph).astype(dtype)
    sin_a = sin.copy()
    sin_a[:, 0::2] *= dtype(-1.0)  # fold the rotate-pair sign into sin
    return cos, sin_a


def build_nc(h_per_core=H_PER_CORE, t=T, n=N, waitsplit=True):
    assert n == 256 and t % 256 == 0
    nt = t // 128   # 128-row t-tiles (16)
    nmac = t // 256  # 256-row macro chunks (8)
    nc = bass.Bass("TRN2", target_bir_lowering=False, debug=False)

    qct = nc.dram_tensor("qct", [h_per_core, n, t], FP16, kind="ExternalInput").ap()
    # natural-layout QR and V pre-arranged as [p, t-tile, n] so each
    # partition line is one fat contiguous DMA stream
    qrn = nc.dram_tensor(
        "qrn", [h_per_core, 128, nt * n], FP16, kind="ExternalInput"
    ).ap()
    v = nc.dram_tensor(
        "v", [h_per_core, 128, nt * n], FP16, kind="ExternalInput"
    ).ap()
    o = nc.dram_tensor("o", [h_per_core, n, t], FP16, kind="ExternalOutput").ap()

    with tile.TileContext(nc) as tc:
        with (
            tc.tile_pool(name="const", bufs=1) as cpool,
            tc.tile_pool(name="q", bufs=2) as qpool,
            tc.tile_pool(name="qn", bufs=2) as qnpool,
            tc.tile_pool(name="vh", bufs=2) as vpool,
            tc.tile_pool(name="strip", bufs=2) as strippool,
            tc.tile_pool(name="msb", bufs=2) as mpool,
            tc.tile_pool(name="ot", bufs=2) as otpool,
            tc.tile_pool(name="sps", bufs=2, space="PSUM") as spool,
            tc.tile_pool(name="ops", bufs=1, space="PSUM") as opool,
            tc.tile_pool(name="mps", bufs=1, space="PSUM") as mpspool,
        ):
            # strict-causal mask in (s, t) orientation: strict-lower mask in
            # the first 128 cols (keep iff free > part), ones after
            mask = cpool.tile([128, 256], F32)
            nc.gpsimd.memset(mask, 1.0)
            nc.gpsimd.affine_select(
                out=mask[:, 0:128],
                in_=mask[:, 0:128],
                compare_op=mybir.AluOpType.is_ge,
                fill=0.0,
                base=-1,
                pattern=[[1, 128]],
                channel_multiplier=-1,
            )
            # persistent M state: one PSUM bank per n1-chunk, accumulated
            # across each head's macro chunks (start=True at macro 0)
            m_ps = [
                mpspool.tile([128, n], F32, name=f"mps{c}") for c in range(2)
            ]

            cp = 0  # copy-engine round robin

            def xcopy(dst, src):
                # gpsimd cannot read PSUM on TRN2, so only DVE + Act rotate
                nonlocal cp
                if cp % 2 == 0:
                    nc.vector.tensor_copy(out=dst, in_=src)
                else:
                    nc.scalar.copy(out=dst, in_=src)
                cp += 1

            def mcopy(dst, src, m):
                # masked (diagonal) copies need tensor_tensor -> DVE only
                nc.vector.tensor_tensor(out=dst, in0=src, in1=m, op=MULT)

            def emit_input_dmas(h):
                # inputs in consumption order: qct on the sync queue,
                # v/qrn on the gpsimd queue, outputs on the scalar queue
                qc = [
                    qpool.tile([128, t], FP16, tag=f"q{c}", name=f"q{c}")
                    for c in range(2)
                ]
                qn = qnpool.tile([128, nt * n], FP16, tag="qn", name="qn")
                vh = vpool.tile([128, nt * n], FP16, tag="vh", name="vh")

                def qseg(lo, hi):
                    # qct on the sync queue
                    for c in range(2):
                        nc.sync.dma_start(
                            out=qc[c][:, lo:hi],
                            in_=qct[h][c * 128:(c + 1) * 128, lo:hi],
                        )

                def tseg(dst, src, G):
                    # v/qrn on the gpsimd queue, qct on sync, outputs on
                    # scalar: three independent DMA queues
                    sl = slice(4 * G * n, 4 * (G + 1) * n)
                    nc.gpsimd.dma_start(out=dst[:, sl], in_=src[h][:, sl])

                qseg(0, 256)
                tseg(vh, v, 0)
                qseg(256, 1024)
                tseg(qn, qrn, 0)
                tseg(vh, v, 1)
                qseg(1024, 2048)
                tseg(qn, qrn, 1)
                tseg(vh, v, 2)
                tseg(qn, qrn, 2)
                tseg(vh, v, 3)
                tseg(qn, qrn, 3)
                return qc, qn, vh

            for h in range(h_per_core):
                qc, qn, vh = emit_input_dmas(h)
                ot_sb = [
                    otpool.tile([128, t], FP16, tag=f"ot{c}", name=f"ot{c}")
                    for c in range(2)
                ]
                prev_msb = None

                for D in range(nmac):
                    t0 = D * 256
                    # ---- intra-chunk masked score strips (S^T, fp16) ----
                    psS = [
                        spool.tile([128, 256], F32, name="ps")
                        for _ in range(2)
                    ]
                    for c in range(2):
                        nc.tensor.matmul(
                            psS[0],
                            lhsT=qc[c][:, t0:t0 + 128],
                            rhs=qc[c][:, t0:t0 + 256],
                            start=(c == 0),
                            stop=(c == 1),
                        )
                        nc.tensor.matmul(
                            psS[1][:, 0:128],
                            lhsT=qc[c][:, t0 + 128:t0 + 256],
                            rhs=qc[c][:, t0 + 128:t0 + 256],
                            start=(c == 0),
                            stop=(c == 1),
                        )
                    s0 = strippool.tile([128, 256], FP16, tag="s0", name="s0")
                    s1 = strippool.tile([128, 128], FP16, tag="s1", name="s1")
                    mcopy(s0, psS[0], mask)
                    mcopy(s1, psS[1][:, 0:128], mask[:, 0:128])

                    # ---- O^T chains: intra (S@V) then inter (QR @ M) ----
                    otg = [
                        opool.tile([128, 256], F32, name=f"otg{b}")
                        for b in range(2)
                    ]
                    for b in range(2):
                        nc.tensor.matmul(
                            otg[b],
                            lhsT=vh[:, 2 * D * n + b * 128:
                                    2 * D * n + b * 128 + 128],
                            rhs=s0,
                            start=True,
                            stop=False,
                        )
                    for b in range(2):
                        nc.tensor.matmul(
                            otg[b][:, 128:256],
                            lhsT=vh[:, (2 * D + 1) * n + b * 128:
                                    (2 * D + 1) * n + b * 128 + 128],
                            rhs=s1,
                            start=False,
                            stop=(D == 0),
                        )
                    if D > 0:
                        for c in range(2):
                            for b in range(2):
                                nc.tensor.matmul(
                                    otg[b],
                                    lhsT=prev_msb[c][:, b * 128:(b + 1) * 128],
                                    rhs=qc[c][:, t0:t0 + 256],
                                    start=False,
                                    stop=(c == 1),
                                )
                    nc.vector.tensor_copy(
                        out=ot_sb[0][:, t0:t0 + 256], in_=otg[0]
                    )
                    nc.scalar.copy(out=ot_sb[1][:, t0:t0 + 256], in_=otg[1])

                    # ---- state update: M += QR_D^T @ V_D (fp32 PSUM) ----
                    for sub in range(2):
                        tt = 2 * D + sub
                        for c in range(2):
                            nc.tensor.matmul(
                                m_ps[c],
                                lhsT=qn[:, tt * n + c * 128:
                                        tt * n + c * 128 + 128],
                                rhs=vh[:, tt * n:(tt + 1) * n],
                                start=(D == 0 and sub == 0),
                                stop=(sub == 1),
                                skip_group_check=True,
                            )
                    if D < nmac - 1:
                        msb = [
                            mpool.tile([128, n], FP16, tag=f"m{c}",
                                       name=f"m{c}")
                            for c in range(2)
                        ]
                        nc.vector.tensor_copy(out=msb[0], in_=m_ps[0])
                        nc.scalar.copy(out=msb[1], in_=m_ps[1])
                        prev_msb = msb

                    if D % 2 == 1:  # drain O^T in 512-col pieces
                        lo = (D - 1) * 256
                        for b in range(2):
                            nc.scalar.dma_start(
                                out=o[h][b * 128:(b + 1) * 128, lo:lo + 512],
                                in_=ot_sb[b][:, lo:lo + 512],
                            )

    if waitsplit:
        _split_overloaded_waits(nc)
    return nc


_NC_CACHE = {}


def get_nc(h_per_core=H_PER_CORE, t=T, n=N):
    key = (h_per_core, t, n)
    if key not in _NC_CACHE:
        _NC_CACHE[key] = build_nc(h_per_core, t, n)
    return _NC_CACHE[key]


def make_in_maps(Q, V, n_cores=N_CORES):
    b, nh, t, n = Q.shape
    h_per_core = (b * nh) // n_cores
    q = np.asarray(Q, dtype=np.float64).reshape(b * nh, t, n)
    vf = np.asarray(V, dtype=np.float32).reshape(b * nh, t, n)
    # RoPE on host (elementwise preprocessing), fp64 for accuracy
    cos, sin_a = rope_tables(t, n, np.float64)
    qsw = np.empty_like(q)
    qsw[..., 0::2] = q[..., 1::2]
    qsw[..., 1::2] = q[..., 0::2]
    qr = q * cos + qsw * sin_a
    qrh = qr.astype(HF)
    # (n, t) layout for the score/inter matmuls
    qct = np.ascontiguousarray(qrh.transpose(0, 2, 1))

    def arrange(x):  # [bh, t, n] -> [bh, 128, (t//128)*n] SBUF layout
        return np.ascontiguousarray(
            x.reshape(b * nh, t // 128, 128, n).transpose(0, 2, 1, 3)
        ).reshape(b * nh, 128, (t // 128) * n)

    qrn = arrange(qrh)
    vb = arrange(vf.astype(HF))
    in_maps = []
    for c in range(n_cores):
        sl = slice(c * h_per_core, (c + 1) * h_per_core)
        in_maps.append(
            {
                "qct": np.ascontiguousarray(qct[sl]),
                "qrn": np.ascontiguousarray(qrn[sl]),
                "v": np.ascontiguousarray(vb[sl]),
            }
        )
    return in_maps


def assemble_output(res, b=B, nh=NH, t=T, n=N, n_cores=N_CORES):
    """Gather per-core O^T (h, n, t) fp16 outputs into (b, nh, t, n) fp32."""
    outs = [res.results[c]["o"] for c in range(n_cores)]
    ot = np.concatenate(outs, axis=0).astype(np.float32)  # (b*nh, n, t)
    return np.ascontiguousarray(ot.transpose(0, 2, 1)).reshape(b, nh, t, n)


def kernel(Q, K, V):
    """Full-input entry point: Q, K, V are (B, NH, T, N) float32 numpy arrays.
    K is unused (the module self-keys attention on rotated Q)."""
    Q = np.asarray(Q)
    V = np.asarray(V)
    b, nh, t, n = Q.shape
    nc = get_nc((b * nh) // N_CORES, t, n)
    in_maps = make_in_maps(Q, V, N_CORES)
    res = None
    last_err = None
    for attempt in range(3):  # retry transient device/runtime failures
        try:
            res = run_bass_kernel_spmd(
                nc, in_maps, core_ids=list(range(N_CORES)), trace=False
            )
            break
        except Exception as e:  # e.g. NRT_EXEC_UNIT_UNRECOVERABLE after a
            last_err = e  # wedged prior run; a clean retry usually recovers
            import time as _time

            _time.sleep(2.0 * (attempt + 1))
    if res is None:
        raise last_err
    return assemble_output(res, b, nh, t, n, N_CORES)
